# revision 68
# baseline (speedup 1.0000x reference)
# Multi-head attention (B=2, S=2048, D=1024, H=16) on 8 TRN2 NeuronCores.
#
# Sharding: core c -> batch b = c//4, head group g = c%4 (4 heads = 256
# features). Each core computes its heads' attention for its batch plus the
# row-parallel slice of the output projection; the host sums the 4 partials
# per batch (the all-reduce) and adds bo.
#
# Phase pipeline: K-projection runs first (qs-blocked, one PSUM tile at a
# time), then Q's first 512-column block, then attention begins; the V
# projection and Q's remaining blocks are interleaved into attention block
# 0's k-chunk loop so the exp pipeline (the ScalarE floor) starts ~25 us
# earlier and no engine idles through a serial projection phase.
#
# Device math per core (layouts transposed so softmax needs no cross-
# partition reduce; all matmul operands fp16, accumulation fp32 in PSUM):
#   qhT[f, s] = wq_g @ q_b^T ; khT, vhT likewise     (fp16 matmuls)
#   vh slots per head: [ones 64 | v 64]              (denominator trick)
#   logitsT[k, q] = khT_h-slices^T @ qhT_h           (K=64, pairs row-packed,
#                                                     T0/T8 tile concurrency)
#   expT = exp(logitsT / 8) * (1 - mask)^T           (ACT exp + DVE mask mult)
#   av[:, q] = vh_slot^T @ expT                      (rows 0:64 = denominator
#                                                     replicated, 64:128 = out)
#   attnN2 = av_out * recip(av_denom)                (both heads packed in one
#                                                     128-part tile; the two
#                                                     cross-partition moves per
#                                                     pair run on ScalarE; the
#                                                     reciprocals must sit at
#                                                     partition base 0)
#   partial[q, D] = attnN2^T @ wo2[pair]             (K=128 packed pairs)
import os
import numpy as np

B, S, DM, H, DEPTH = 2, 2048, 1024, 16, 64
NCORES = 8
GROUPS = 4            # head-groups per batch == cores per batch
HG = H // GROUPS      # heads per core
FS = HG * DEPTH       # features per core
QC = 512              # q-block (matmul free dim)
NQC = S // QC
NKC = S // 128        # k chunks
PAIRS = HG // 2
CCH = DM // 128       # contraction chunks for the projections

_CACHE = {}


def _build():
    import concourse.tile as tile
    from concourse import bacc, mybir

    dt = mybir.dt
    f32, f16 = dt.float32, dt.float16
    Act = mybir.ActivationFunctionType

    nc = bacc.Bacc("TRN2", target_bir_lowering=False, debug=False,
                   num_devices=NCORES)

    xq = nc.dram_tensor("xq", [DM, S], f16, kind="ExternalInput").ap()
    xk = nc.dram_tensor("xk", [DM, S], f16, kind="ExternalInput").ap()
    xv = nc.dram_tensor("xv", [DM, S], f16, kind="ExternalInput").ap()
    wqd = nc.dram_tensor("wq", [128, CCH, FS], f16, kind="ExternalInput").ap()
    wkd = nc.dram_tensor("wk", [128, CCH, FS], f16, kind="ExternalInput").ap()
    wvd = nc.dram_tensor("wv", [128, CCH, FS], f16, kind="ExternalInput").ap()
    wod = nc.dram_tensor("wo", [PAIRS, 128, DM], f16, kind="ExternalInput").ap()
    m01 = nc.dram_tensor("m01", [S, S], f16, kind="ExternalInput").ap()
    bqd = nc.dram_tensor("bq", [128, 2], f32, kind="ExternalInput").ap()
    bkd = nc.dram_tensor("bk", [128, 2], f32, kind="ExternalInput").ap()
    out = nc.dram_tensor("part", [S, DM], f16, kind="ExternalOutput").ap()

    with tile.TileContext(nc) as tc:
        with (
            tc.tile_pool(name="xp", bufs=16) as xp,
            tc.tile_pool(name="wp", bufs=2) as wp,
            tc.tile_pool(name="wvp", bufs=1) as wvp,
            tc.tile_pool(name="wop", bufs=2) as wop,
            tc.tile_pool(name="qk", bufs=4) as qkp,
            tc.tile_pool(name="vh", bufs=16) as vp,
            tc.tile_pool(name="mk", bufs=32) as mkp,
            tc.tile_pool(name="ex", bufs=5) as exp_p,
            tc.tile_pool(name="exm", bufs=8) as exm_p,
            tc.tile_pool(name="au", bufs=4) as aup,
            tc.tile_pool(name="rc", bufs=4) as rcp,
            tc.tile_pool(name="an", bufs=4) as anp,
            tc.tile_pool(name="os", bufs=4) as osp,
            tc.tile_pool(name="cst", bufs=4) as cst,
            tc.tile_pool(name="ps", bufs=4, space="PSUM") as psp,
        ):
            def big():
                return psp.tile([128, 2, QC], f32, tag="big", name="big")

            # weights in use-order: K first (its projection runs first)
            wk_t = wp.tile([128, CCH, FS], f16, tag="w", name="w")
            nc.sync.dma_start(wk_t[:], wkd[:])
            bk_t = cst.tile([128, 2], f32, tag="bias", name="bias")
            nc.sync.dma_start(bk_t[:], bkd[:])
            # vh tiles pre-allocated; ones slots memset early on the then-
            # idle DVE.  Slot layout per head: [ones 64 | v 64] -> av rows
            # 0:64 = denominator (replicated), rows 64:128 = head output.
            vh = [vp.tile([128, HG, 128], f16, tag="vh", name="vh")
                  for _ in range(NKC)]
            for kr in range(NKC):
                nc.vector.memset(vh[kr][:, :, 0:64], 1.0)

            qhT = [qkp.tile([128, S], f16, tag="qk", name="qk") for _ in range(2)]
            khT = [qkp.tile([128, S], f16, tag="qk", name="qk") for _ in range(2)]

            # ---- K projection, qs-blocked (one rotating PSUM tile) ----
            xk_t = []
            for qs in range(4):
                qsl = slice(QC * qs, QC * (qs + 1))
                pk = big()
                for c in range(CCH):
                    if qs == 0:
                        t = xp.tile([128, S], f16, tag="x", name="x")
                        nc.sync.dma_start(t[:], xk[128 * c:128 * (c + 1), :])
                        xk_t.append(t)
                    for m in range(2):
                        nc.tensor.matmul(
                            pk[:, m, :],
                            lhsT=wk_t[:, c, 128 * m:128 * (m + 1)],
                            rhs=xk_t[c][:, qsl],
                            start=(c == 0), stop=(c == CCH - 1))
                for m in range(2):
                    nc.scalar.add(khT[m][:, qsl], pk[:, m, :], bk_t[:, m:m + 1])

            # Q weights next: q_qs(0) gates the attention start, so its
            # DMAs must not queue behind the (deadline-relaxed) V loads
            wq_t = wp.tile([128, CCH, FS], f16, tag="w", name="w")
            nc.sync.dma_start(wq_t[:], wqd[:])
            bq_t = cst.tile([128, 2], f32, tag="bias", name="bias")
            nc.sync.dma_start(bq_t[:], bqd[:])

            # ---- Q projection, one qs block at a time; blocks 1-3 are
            # emitted from inside attention block 0 ----
            xq_t = []

            def q_qs(qs):
                qsl = slice(QC * qs, QC * (qs + 1))
                pq = big()
                for c in range(CCH):
                    if qs == 0:
                        t = xp.tile([128, S], f16, tag="x", name="x")
                        nc.sync.dma_start(t[:], xq[128 * c:128 * (c + 1), :])
                        xq_t.append(t)
                    for m in range(2):
                        nc.tensor.matmul(
                            pq[:, m, :],
                            lhsT=wq_t[:, c, 128 * m:128 * (m + 1)],
                            rhs=xq_t[c][:, qsl],
                            start=(c == 0), stop=(c == CCH - 1))
                for m in range(2):
                    # block 0 gates attention start -> ScalarE (idle); the
                    # rest run during attention -> DVE, off the exp path
                    if qs == 0:
                        nc.scalar.add(qhT[m][:, qsl], pq[:, m, :],
                                      bq_t[:, m:m + 1])
                    else:
                        nc.vector.tensor_scalar_add(qhT[m][:, qsl],
                                                    pq[:, m, :],
                                                    bq_t[:, m:m + 1])

            q_qs(0)

            # V inputs (shares the x pool; k slices free up as k-proj ends)
            wv_t = wvp.tile([128, CCH, FS], f16, tag="w", name="w")
            nc.sync.dma_start(wv_t[:], wvd[:])
            xv_t = []
            for c in range(CCH):
                t = xp.tile([128, S], f16, tag="x", name="x")
                nc.sync.dma_start(t[:], xv[128 * c:128 * (c + 1), :])
                xv_t.append(t)

            wo_t = []
            for p in range(PAIRS):
                t = wop.tile([128, DM], f16, tag="wo", name="wo")
                nc.sync.dma_start(t[:], wod[p])
                wo_t.append(t)

            # ---- V projection, per k-row chunk; kr 0-3 up front, the rest
            # interleaved into attention block 0 (vh[kr] is first needed by
            # the AV matmul of k-chunk kr, which trails logits by 4) ----
            def v_kr(kr):
                pv = psp.tile([128, 256], f32, tag="big", name="big")
                for c in range(CCH):
                    nc.tensor.matmul(
                        pv[:], lhsT=xv_t[c][:, 128 * kr:128 * (kr + 1)],
                        rhs=wv_t[:, c, :],
                        start=(c == 0), stop=(c == CCH - 1))
                nc.vector.tensor_copy(
                    vh[kr][:, :, 64:128],
                    pv.rearrange("p (h d) -> p h d", d=DEPTH))



            # ---- attention + output projection, per q-block ----
            def emit_wo_qm(qcb, attnN2, qm):
                row = slice(128 * (4 * qcb + qm), 128 * (4 * qcb + qm + 1))
                po = big()
                for dn in range(2):
                    dsl = slice(512 * dn, 512 * (dn + 1))
                    for p in range(PAIRS):
                        nc.tensor.matmul(
                            po[:, dn, :],
                            lhsT=attnN2[p][:, 128 * qm:128 * (qm + 1)],
                            rhs=wo_t[p][:, dsl],
                            start=(p == 0), stop=(p == PAIRS - 1))
                ot = osp.tile([128, 2, 512], f16, tag="os", name="os")
                nc.vector.tensor_copy(ot[:], po[:])
                nc.sync.dma_start(
                    out[row, :].rearrange("p (o q) -> p o q", o=2), ot[:])

            prev_wo = None
            for qcb in range(NQC):
                qsl = slice(QC * qcb, QC * (qcb + 1))
                mk = []
                for kc in range(NKC):
                    t = mkp.tile([128, QC], f16, tag="mk", name="mk")
                    nc.sync.dma_start(
                        t[:], m01[128 * kc:128 * (kc + 1), qsl])
                    mk.append(t)

                av2 = [big() for _ in range(PAIRS)]   # halves = heads A/B

                def emit_av(pair, dk, exm2, av2=av2):
                    for half in range(2):
                        nc.tensor.matmul(
                            av2[pair][:, half, :],
                            lhsT=vh[dk][:, 2 * pair + half, :],
                            rhs=exm2[:, half, :],
                            start=(dk == 0), stop=(dk == NKC - 1),
                            skip_group_check=True)

                def normalize(pair, av2=av2):
                    # av2[pair][0:64, half] = denominator (replicated),
                    # av2[pair][64:128, half] = head output.  DVE lanes are
                    # partition-locked and reciprocal_approx only works at
                    # partition base 0, so the two cross-partition moves
                    # per pair go through ScalarE.
                    au2 = aup.tile([128, QC], f32, tag="au", name="au")
                    rc2 = rcp.tile([128, QC], f32, tag="rc", name="rc")
                    nc.scalar.copy(au2[0:64, :], av2[pair][64:128, 0, :])
                    nc.vector.tensor_copy(au2[64:128, :],
                                          av2[pair][64:128, 1, :])
                    nc.vector.reciprocal_approx_fast(
                        rc2[0:64, :], av2[pair][0:64, 0, :])
                    rcb = rcp.tile([128, QC], f32, tag="rc", name="rc")
                    nc.vector.reciprocal_approx_fast(
                        rcb[0:64, :], av2[pair][0:64, 1, :])
                    nc.scalar.copy(rc2[64:128, :], rcb[0:64, :])
                    an2 = anp.tile([128, QC], f16, tag="an", name="an")
                    nc.vector.tensor_mul(an2[:], au2[:], rc2[:])
                    return an2

                # Both pairs' pipelines interleaved; AV trails logits by 4
                # k-chunks so the PE never stalls on the exp/mask pipeline.
                # The previous q-block's output projection is spread over
                # kc 6/8/10/12; in block 0 the V projection and Q blocks
                # 1-3 are interleaved instead.
                pend = {p: [] for p in range(PAIRS)}
                for kc in range(NKC):
                    ksl = slice(128 * kc, 128 * (kc + 1))
                    for pair in range(PAIRS):
                        lg2 = big()
                        for half in range(2):
                            psl = slice(64 * half, 64 * (half + 1))
                            nc.tensor.matmul(
                                lg2[:, half, :],
                                lhsT=khT[pair][psl, ksl],
                                rhs=qhT[pair][psl, qsl],
                                start=True, stop=True)
                        ex2 = exp_p.tile([128, 2, QC], f16, tag="ex", name="ex")
                        nc.scalar.activation(
                            ex2[:], lg2[:], Act.Exp, scale=0.125)
                        exm2 = exm_p.tile([128, 2, QC], f16, tag="exm",
                                          name="exm")
                        mbc = (mk[kc][:].rearrange("p (o q) -> p o q", o=1)
                               .to_broadcast((128, 2, QC)))
                        nc.vector.tensor_mul(exm2[:], ex2[:], mbc)
                        pend[pair].append((pair, kc, exm2))
                        if len(pend[pair]) > 3:
                            p_, dk, dexm = pend[pair].pop(0)
                            emit_av(p_, dk, dexm)
                    if qcb == 0:
                        if kc < 2:
                            v_kr(2 * kc)
                            v_kr(2 * kc + 1)
                        elif kc < 14:
                            v_kr(kc + 2)
                        if kc in (3, 7, 11):
                            q_qs((kc + 1) // 4)
                    if prev_wo is not None and kc in (6, 8, 10, 12):
                        pq, attnN2 = prev_wo
                        emit_wo_qm(pq, attnN2, (kc - 6) // 2)
                        if kc == 12:
                            prev_wo = None
                attnN2 = []
                for p in range(PAIRS):
                    for p_, dk, dexm in pend[p]:
                        emit_av(p_, dk, dexm)
                    attnN2.append(normalize(p))
                prev_wo = (qcb, attnN2)
            pq, attnN2 = prev_wo
            for qm in range(4):
                emit_wo_qm(pq, attnN2, qm)

    nc.compile()
    return nc


def _get_program():
    if "nc" not in _CACHE:
        _CACHE["nc"] = _build()
    return _CACHE["nc"]


def _in_maps(q, k, v, mask, wq, bq, wk, bk, wv, bv, wo, bo):
    q = np.asarray(q, np.float32)
    k = np.asarray(k, np.float32)
    v = np.asarray(v, np.float32)
    mask = np.asarray(mask, np.float32)
    wq = np.asarray(wq, np.float32)
    wk = np.asarray(wk, np.float32)
    wv = np.asarray(wv, np.float32)
    wo = np.asarray(wo, np.float32)
    bq = np.asarray(bq, np.float32)
    bk = np.asarray(bk, np.float32)
    bv = np.asarray(bv, np.float32)
    assert np.all(bv == 0.0), "nonzero bv not supported by this kernel"

    def wdev(w, cols):
        # [128, CCH, FS] layout: partition p, contraction chunk c holds
        # dram row 128*c + p of w[cols].T
        wT = np.ascontiguousarray(w[cols].T).astype(np.float16)
        return np.ascontiguousarray(
            wT.reshape(CCH, 128, FS).transpose(1, 0, 2))

    maps = []
    xqT = [np.ascontiguousarray(q[b].T).astype(np.float16) for b in range(B)]
    xkT = [np.ascontiguousarray(k[b].T).astype(np.float16) for b in range(B)]
    xvT = [np.ascontiguousarray(v[b].T).astype(np.float16) for b in range(B)]
    m01 = [np.ascontiguousarray((1.0 - mask[b, 0]).T).astype(np.float16)
           for b in range(B)]
    for c in range(NCORES):
        b, g = divmod(c, GROUPS)
        cols = slice(FS * g, FS * (g + 1))
        maps.append({
            "xq": xqT[b], "xk": xkT[b], "xv": xvT[b],
            "wq": wdev(wq, cols),
            "wk": wdev(wk, cols),
            "wv": wdev(wv, cols),
            "wo": np.ascontiguousarray(
                wo[:, cols].T.reshape(PAIRS, 128, DM)).astype(np.float16),
            "m01": m01[b],
            "bq": np.ascontiguousarray(bq[cols].reshape(2, 128).T),
            "bk": np.ascontiguousarray(bk[cols].reshape(2, 128).T),
        })
    return maps


def _run(maps, trace=False):
    from concourse.bass_utils import run_bass_kernel_spmd
    nc = _get_program()
    kwargs = {}
    if trace:
        kwargs = dict(trace=True, tmpdir=os.environ.get("KERNEL_TRACE_DIR"))
    return run_bass_kernel_spmd(nc, maps, list(range(NCORES)), **kwargs)


def kernel(q, k, v, mask, wq, bq, wk, bk, wv, bv, wo, bo):
    maps = _in_maps(q, k, v, mask, wq, bq, wk, bk, wv, bv, wo, bo)
    res = _run(maps)
    parts = [res.results[c]["part"].astype(np.float32) for c in range(NCORES)]
    bo = np.asarray(bo, np.float32)
    outb = [parts[GROUPS * b] + parts[GROUPS * b + 1]
            + parts[GROUPS * b + 2] + parts[GROUPS * b + 3] + bo
            for b in range(B)]
    return np.stack(outb, 0).astype(np.float32)
